# revision 20
# baseline (speedup 1.0000x reference)
"""Trainium2 Bass kernel for nn_EventWarping (contrast-maximization event
warping loss).

Strategy (data-parallel over batch, one NeuronCore per batch element):
  The core op is a bilinear scatter-add of N=262144 warped events into a
  256x256 image (4 images per warp: pos/neg polarity x {weight, weight*ts}).
  We use the TensorEngine outer-product histogram: for a chunk of 128
  events build per-event y-tent rows (lhsT) and x-tent rows (rhs) and
  accumulate image += lhsT^T @ rhs into PSUM.

  v2 over the original baseline:
  - Events are split by polarity on the host, so each chunk feeds exactly
    one polarity's images: 4 matmuls per chunk instead of 8.
  - Tents are built as t(d) = relu(1 - |iota - w|) directly from the raw
    warped coordinate (no floor/frac preprocessing on the critical path).
    We actually compute the NEGATED tent  tn = min(|iota - w|, 1) - 1  in a
    single fused tensor_scalar; lhsT and rhs are both negated so the
    matmul products are unchanged.
  - |iota - w| comes from the Scalar engine (Abs activation with a
    per-partition f32 bias) for 3 of the 4 warp-coords, and from a
    bf16 int/frac broadcast-subtract chain on DVE+GpSimd for the 4th,
    balancing the three element-wise engines.
  - 16-chunk unrolled hardware loop (the For_i cross-engine barrier costs
    ~1.6us per iteration).
  Epilogue computes sum((num/(den+eps))^2)/mt^2/nonzero_px per warp plus
  the Charbonnier flow-smoothness term on-device; host sums the 8
  per-core partial losses.
"""

import sys

if "/opt/trn_rl_repo" not in sys.path:
    sys.path.insert(0, "/opt/trn_rl_repo")

from contextlib import ExitStack

import ml_dtypes
import numpy as np

import concourse.bacc as bacc
import concourse.bass as bass
import concourse.mybir as mybir
from concourse.tile import TileContext

F32 = mybir.dt.float32
BF16 = mybir.dt.bfloat16
I32 = mybir.dt.int32
AL = mybir.AluOpType
ACTF = mybir.ActivationFunctionType

P = 128
RES = 256
NPIX = RES * RES
EPS = 1e-9
FLOW_TEMP_REG = 1e-3
import os as _os
U = int(_os.environ.get("EVW_U", "16"))  # chunks per hardware-loop iteration
_E1E2_ENGINE = _os.environ.get("EVW_E1E2", "gpsimd")
CB = 512  # static block size (dynamic AP offsets are register-limited)
PAD_POS = 1.0e6  # padding sentinel coordinate (far outside the grid)


def _emit(tc, ev, iotas, vecb, loss_out, C, mt, img_out=None):
    """C = chunks per polarity group; ev rows 5g+k hold group g's field k
    (k: 0=ts 1=y 2=x 3=fy 4=fx), each [1, C*128]."""
    nc = tc.nc
    stk = ExitStack()

    const_pool = stk.enter_context(tc.tile_pool(name="const", bufs=1))
    iota = const_pool.tile([P, 256], BF16)
    nc.sync.dma_start(iota, iotas[:, 0:256])
    ones = const_pool.tile([P, 1], F32)
    nc.gpsimd.memset(ones, 1.0)
    zk = const_pool.tile([1, 640], BF16)
    nc.gpsimd.memset(zk, 0.0)
    vtile = const_pool.tile([1, 32], F32)
    nc.sync.dma_start(vtile, vecb)

    # ---- preprocessing: per group, per warp ----
    # persistent per-(group,warp) tiles:
    #   nwy (f32)   : -warped_y            (ACT Abs bias)
    #   tw  (bf16)  : temporal weight (ts for warp0, mt-ts for warp1)
    # warp0 x: nwx0 (f32) for ACT; warp1 x: x1b/fx1b (bf16 int+frac) for DVE/GP
    fld_pool = stk.enter_context(tc.tile_pool(name="fld", bufs=1))
    grp = []
    raw_stk = ExitStack()
    raw_pool = raw_stk.enter_context(tc.tile_pool(name="raw", bufs=1))
    for g in (0, 1):
        def load_field(k):
            t = raw_pool.tile([P, C], F32, tag=f"raw{g}{k}", name=f"raw{g}{k}")
            nc.sync.dma_start(
                t, ev[5 * g + k : 5 * g + k + 1, :].rearrange("o (p c) -> (o p) c", p=P)
            )
            return t

        ts_t, y_t, x_t, fy_t, fx_t = [load_field(k) for k in range(5)]
        q_t = raw_pool.tile([P, C], F32, tag=f"q{g}", name=f"q{g}")
        nc.vector.tensor_scalar(q_t, ts_t, -1.0, float(mt), AL.mult, AL.add)

        d = {}
        scr = raw_pool.tile([P, C], F32, tag=f"scr{g}", name=f"scr{g}")
        # warp0: w = coord + (mt-ts)*flow ; warp1: w = coord - ts*flow
        for w, (mult_t, sign) in enumerate(((q_t, 1.0), (ts_t, -1.0))):
            for cname, coord, flow in (("y", y_t, fy_t), ("x", x_t, fx_t)):
                if w == 1 and cname == "x":
                    continue  # handled below via int/frac path
                nw = fld_pool.tile([P, C], F32, tag=f"nw{g}{w}{cname}",
                                   name=f"nw{g}{w}{cname}")
                nc.vector.tensor_tensor(out=scr, in0=mult_t, in1=flow, op=AL.mult)
                if sign > 0:
                    nc.vector.tensor_tensor(out=nw, in0=scr, in1=coord, op=AL.add)
                else:
                    nc.vector.tensor_tensor(out=nw, in0=coord, in1=scr, op=AL.subtract)
                    nc.vector.tensor_scalar(nw, nw, -1.0, None, AL.mult)
                    d[f"nw{w}{cname}"] = nw
                    continue
                nc.vector.tensor_scalar(nw, nw, -1.0, None, AL.mult)
                d[f"nw{w}{cname}"] = nw
        # warp1 x: split into bf16 integer + fraction
        wx1 = raw_pool.tile([P, C], F32, tag=f"wx1{g}", name=f"wx1{g}")
        nc.vector.tensor_tensor(out=scr, in0=ts_t, in1=fx_t, op=AL.mult)
        nc.vector.tensor_tensor(out=wx1, in0=x_t, in1=scr, op=AL.subtract)
        xi = raw_pool.tile([P, C], I32, tag=f"xi{g}", name=f"xi{g}")
        nc.vector.tensor_copy(out=xi, in_=wx1)
        x0f = raw_pool.tile([P, C], F32, tag=f"x0f{g}", name=f"x0f{g}")
        nc.vector.tensor_copy(out=x0f, in_=xi)
        x1b = fld_pool.tile([P, C], BF16, tag=f"x1b{g}", name=f"x1b{g}")
        nc.vector.tensor_copy(out=x1b, in_=x0f)
        nc.vector.tensor_tensor(out=scr, in0=wx1, in1=x0f, op=AL.subtract)
        fx1b = fld_pool.tile([P, C], BF16, tag=f"fx1b{g}", name=f"fx1b{g}")
        nc.vector.tensor_copy(out=fx1b, in_=scr)
        tw0b = fld_pool.tile([P, C], BF16, tag=f"tw0b{g}", name=f"tw0b{g}")
        nc.vector.tensor_copy(out=tw0b, in_=ts_t)
        tw0f = fld_pool.tile([P, C], F32, tag=f"tw0f{g}", name=f"tw0f{g}")
        nc.vector.tensor_copy(out=tw0f, in_=ts_t)
        d["tw0f"] = tw0f
        tw1b = fld_pool.tile([P, C], BF16, tag=f"tw1b{g}", name=f"tw1b{g}")
        nc.vector.tensor_copy(out=tw1b, in_=q_t)
        d["tw0b"] = tw0b
        d["tw1b"] = tw1b
        d["x1b"] = x1b
        d["fx1b"] = fx1b
        grp.append(d)
    raw_stk.close()

    # ---- PSUM images: banks[g][w][h] = [den(256) | num(256)] for y-half h
    psum_pool = tc.tile_pool(name="psum", bufs=1, space="PSUM")
    psum = psum_pool.__enter__()
    banks = [
        [
            [
                psum.tile([P, 512], F32, tag=f"B{g}{w}{h}", name=f"B{g}{w}{h}")
                for h in (0, 1)
            ]
            for w in (0, 1)
        ]
        for g in (0, 1)
    ]
    zl = zk[0:1, 0:128]
    zr = zk[0:1, 128:640]
    for g in (0, 1):
        for w in (0, 1):
            for h in (0, 1):
                nc.tensor.matmul(
                    out=banks[g][w][h][:], lhsT=zl, rhs=zr, start=True, stop=False
                )

    loop_pool = stk.enter_context(tc.tile_pool(name="loop", bufs=4))

    def stage_biases(g, i, base, span):
        """ACT bias APs cannot carry dynamic offsets (silently read garbage),
        so copy this iteration's U bias columns into static staging tiles."""
        d = grp[g]
        stg = {}
        for key in ("nw0y", "nw1y", "nw0x", "tw0f"):
            t = loop_pool.tile([P, U], F32, tag=f"stg_{key}", name=f"stg_{key}")
            nc.vector.tensor_copy(
                out=t, in_=d[key][:, base : base + span][:, bass.ds(i, U)]
            )
            stg[key] = t
        return stg

    def chunk_produce(g, i, base, span, u, stg):
        """Producer ops (ACT + GpSimd) for chunk u of this iteration."""
        d = grp[g]

        def col(t):
            return t[:, base : base + span][:, bass.ds(i, U)][:, u : u + 1]

        # ACT: |iota - w| for warp0 y, warp1 y, warp0 x -> one contiguous tile
        ayx = loop_pool.tile([P, 1024], BF16, tag="ayx", name="ayx")
        nc.scalar.activation(
            ayx[:, 0:256], iota, ACTF.Abs, bias=stg["nw0y"][:, u : u + 1], scale=1.0
        )
        nc.scalar.activation(
            ayx[:, 256:512], iota, ACTF.Abs, bias=stg["nw1y"][:, u : u + 1], scale=1.0
        )
        nc.scalar.activation(
            ayx[:, 512:768], iota, ACTF.Abs, bias=stg["nw0x"][:, u : u + 1], scale=1.0
        )
        # GpSimd: warp1 x  e = (iota - x0) - fx  via bf16 int/frac broadcasts
        e1 = loop_pool.tile([P, 256], BF16, tag="e1", name="e1")
        nc.gpsimd.tensor_tensor(
            out=e1, in0=iota, in1=col(d["x1b"]).to_broadcast((P, 256)), op=AL.subtract
        )
        e2 = loop_pool.tile([P, 256], BF16, tag="e2", name="e2")
        nc.gpsimd.tensor_tensor(
            out=e2, in0=e1, in1=col(d["fx1b"]).to_broadcast((P, 256)), op=AL.subtract
        )
        return {"ayx": ayx, "e2": e2, "u": u}

    def chunk_consume(g, i, base, span, pr, stg):
        d = grp[g]
        u = pr["u"]

        def col(t):
            return t[:, base : base + span][:, bass.ds(i, U)][:, u : u + 1]

        # w1-x abs lands in the 4th slot of the ACT tile, then ONE DVE op
        # finishes all four tents: tn = min(d,1) - 1 (negated).
        # TQ layout: [tny0 | tny1 | r00 | r10 | r01 | r11]
        ayx = pr["ayx"]
        nc.vector.scalar_tensor_tensor(
            ayx[:, 768:1024], pr["e2"], -1.0, pr["e2"], AL.mult, AL.max
        )
        TQ = loop_pool.tile([P, 1536], BF16, tag="TQ", name="TQ")
        nc.vector.tensor_scalar(TQ[:, 0:1024], ayx, 1.0, 1.0, AL.min, AL.subtract)
        tny1 = TQ[:, 256:512]
        nc.vector.tensor_tensor(
            out=TQ[:, 1024:1280],
            in0=TQ[:, 512:768],
            in1=col(d["tw0b"]).to_broadcast((P, 256)),
            op=AL.mult,
        )
        nc.gpsimd.tensor_tensor(
            out=TQ[:, 1280:1536],
            in0=TQ[:, 768:1024],
            in1=col(d["tw1b"]).to_broadcast((P, 256)),
            op=AL.mult,
        )
        # rhs: [den | num] as two 256-col blocks 512 apart
        r0 = TQ[:, 512:1536].rearrange("p (b c) -> p b c", b=4)[:, ::2, :]
        r1 = TQ[:, 768:1536].rearrange("p (b c) -> p b c", b=3)[:, ::2, :]

        for h in (0, 1):
            nc.tensor.matmul(
                out=banks[g][0][h][:],
                lhsT=TQ[:, h * 128 : (h + 1) * 128],
                rhs=r0,
                start=False,
                stop=False,
            )
        for h in (0, 1):
            nc.tensor.matmul(
                out=banks[g][1][h][:],
                lhsT=tny1[:, h * 128 : (h + 1) * 128],
                rhs=r1,
                start=False,
                stop=False,
            )

    for g in (0, 1):
        for b in range(0, C, CB):
            span = min(CB, C - b)
            with tc.For_i(0, span, U) as i:
                stg = stage_biases(g, i, b, span)
                prev = None
                for u in range(U):
                    pr = chunk_produce(g, i, b, span, u, stg)
                    if prev is not None:
                        chunk_consume(g, i, b, span, prev, stg)
                    prev = pr
                chunk_consume(g, i, b, span, prev, stg)

    for g in (0, 1):
        for w in (0, 1):
            for h in (0, 1):
                nc.tensor.matmul(
                    out=banks[g][w][h][:], lhsT=zl, rhs=zr, start=False, stop=True
                )

    if img_out is not None:
        with tc.tile_pool(name="dump", bufs=1) as dump_pool:
            stage = dump_pool.tile([P, 512], F32)
            k = 0
            for g in (0, 1):
                for w in (0, 1):
                    for h in (0, 1):
                        nc.vector.tensor_copy(out=stage, in_=banks[g][w][h][:])
                        nc.sync.dma_start(
                            img_out[k * P : (k + 1) * P, :], stage
                        )
                        k += 1

    # ---- epilogue (same math as reference) ----
    epi_pool = stk.enter_context(tc.tile_pool(name="epi", bufs=1))
    rows = epi_pool.tile([P, 4], F32)
    den = epi_pool.tile([P, 256], F32, tag="den")
    num = epi_pool.tile([P, 256], F32, tag="num")
    rec = epi_pool.tile([P, 256], F32, tag="rec")
    for w in (0, 1):
        SQ = epi_pool.tile([P, 256], F32, tag=f"SQ{w}", name=f"SQ{w}")
        Z = epi_pool.tile([P, 256], F32, tag=f"Z{w}", name=f"Z{w}")
        nc.vector.memset(SQ, 0.0)
        nc.vector.memset(Z, 0.0)
        for h in (0, 1):
            Uh, Sh = banks[0][w][h], banks[1][w][h]
            for img in (Uh, Sh):
                nc.vector.tensor_scalar(den, img[:, 0:256], EPS, None, AL.add)
                nc.vector.reciprocal(rec, den)
                nc.vector.tensor_tensor(
                    out=num, in0=img[:, 256:512], in1=rec, op=AL.mult
                )
                nc.vector.tensor_tensor(out=num, in0=num, in1=num, op=AL.mult)
                nc.vector.tensor_tensor(out=SQ, in0=SQ, in1=num, op=AL.add)
            # nonzero-pixel count uses iwe_pos + iwe_neg
            # (only one tensor_tensor input may come from PSUM -> stage S)
            nc.vector.tensor_copy(out=rec, in_=Sh[:, 0:256])
            nc.vector.tensor_tensor(out=den, in0=Uh[:, 0:256], in1=rec, op=AL.add)
            nc.vector.tensor_scalar(den, den, 0.0, None, AL.is_equal)
            nc.vector.tensor_tensor(out=Z, in0=Z, in1=den, op=AL.add)
        nc.vector.tensor_reduce(
            out=rows[:, 2 * w : 2 * w + 1], in_=SQ, axis=mybir.AxisListType.X, op=AL.add
        )
        nc.vector.tensor_reduce(
            out=rows[:, 2 * w + 1 : 2 * w + 2],
            in_=Z,
            axis=mybir.AxisListType.X,
            op=AL.add,
        )

    psum_pool.__exit__(None, None, None)

    with tc.tile_pool(name="psum2", bufs=1, space="PSUM") as psum2:
        red = psum2.tile([1, 4], F32)
        nc.tensor.matmul(out=red[:], lhsT=ones[:], rhs=rows[:], start=True, stop=True)
        scal = epi_pool.tile([1, 4], F32)
        nc.vector.tensor_copy(out=scal, in_=red[:])

    lt = epi_pool.tile([1, 1], F32)
    nc.vector.memset(lt, 0.0)
    t1 = epi_pool.tile([1, 1], F32)
    t2 = epi_pool.tile([1, 1], F32)
    for w in (0, 1):
        # t1 = 65536 - zero_count  (the reference's +EPS is an f32 no-op here)
        nc.vector.tensor_scalar(
            t1, scal[0:1, 2 * w + 1 : 2 * w + 2], -1.0, float(NPIX), AL.mult, AL.add
        )
        nc.vector.reciprocal(t2, t1)
        nc.vector.tensor_scalar(
            t1, scal[0:1, 2 * w : 2 * w + 1], 1.0 / (mt * mt), None, AL.mult
        )
        nc.vector.scalar_tensor_tensor(lt, t1, t2, lt, AL.mult, AL.add)

    # Charbonnier temporal-smoothness on vector_list
    d24 = epi_pool.tile([1, 24], F32)
    nc.vector.tensor_tensor(
        out=d24, in0=vtile[0:1, 0:24], in1=vtile[0:1, 8:32], op=AL.subtract
    )
    epsb = epi_pool.tile([1, 1], F32)
    nc.vector.memset(epsb, EPS)
    nc.scalar.activation(d24, d24, ACTF.Square)
    nc.scalar.activation(d24, d24, ACTF.Sqrt, bias=epsb[0:1, 0:1])
    ch = epi_pool.tile([1, 1], F32)
    nc.vector.tensor_reduce(out=ch, in_=d24, axis=mybir.AxisListType.X, op=AL.add)
    nc.vector.scalar_tensor_tensor(lt, ch, FLOW_TEMP_REG / 24.0, lt, AL.mult, AL.add)

    nc.sync.dma_start(loss_out, lt[:])
    stk.close()


def _build(C, mt, num_devices=8, dump_images=False):
    nc = bacc.Bacc(
        "TRN2", target_bir_lowering=False, debug=False, num_devices=num_devices
    )
    ev = nc.dram_tensor("ev", [10, C * P], F32, kind="ExternalInput")
    iotas = nc.dram_tensor("iotas", [P, 256], BF16, kind="ExternalInput")
    vecb = nc.dram_tensor("vecb", [1, 32], F32, kind="ExternalInput")
    loss = nc.dram_tensor("loss", [1, 1], F32, kind="ExternalOutput")
    img = (
        nc.dram_tensor("img", [8 * P, 512], F32, kind="ExternalOutput")
        if dump_images
        else None
    )
    with TileContext(nc) as tc:
        _emit(tc, ev.ap(), iotas.ap(), vecb.ap(), loss.ap(), C, mt,
              img_out=img.ap() if img is not None else None)
    nc.compile()
    return nc


def _host_iotas():
    a = np.arange(256, dtype=np.float32)
    return np.tile(a[None, :], (P, 1)).astype(ml_dtypes.bfloat16)


def _pack_inputs(event_list, flow, vector_list, NP):
    B = event_list.shape[0]
    iot = _host_iotas()
    maps = []
    for b in range(B):
        ts = event_list[b, :, 0]
        y = event_list[b, :, 1]
        x = event_list[b, :, 2]
        p = event_list[b, :, 3]
        fy = flow[b, :, 0]
        fx = flow[b, :, 1]
        ev = np.zeros((10, NP), np.float32)
        for g, mask in enumerate((p > 0, p <= 0)):
            cnt = int(mask.sum())
            ev[5 * g + 0, :cnt] = ts[mask]
            ev[5 * g + 1, :cnt] = y[mask]
            ev[5 * g + 2, :cnt] = x[mask]
            ev[5 * g + 3, :cnt] = fy[mask]
            ev[5 * g + 4, :cnt] = fx[mask]
            # padding: coordinate far outside the grid, zero flow/weight
            ev[5 * g + 1, cnt:] = PAD_POS
            ev[5 * g + 2, cnt:] = PAD_POS
        # device rearrange views each [1, NP] row as [128, C] row-major
        # (event e -> partition e//C, column e%C), identically for all
        # fields, so events are just permuted across chunks
        maps.append({"ev": ev, "iotas": iot,
                     "vecb": np.ascontiguousarray(
                         vector_list[b].reshape(1, 32), dtype=np.float32)})
    return maps


_NC_CACHE = {}
_RUN_KWARGS = {}  # test harness may set {"trace": True, "tmpdir": ...}
_LAST_RESULT = None


def kernel(event_list, flow, pol_mask, vector_list, max_ts):
    global _LAST_RESULT
    from concourse.bass_utils import run_bass_kernel_spmd

    event_list = np.asarray(event_list)
    flow = np.asarray(flow)
    vector_list = np.asarray(vector_list)
    B, N, _ = event_list.shape
    mt = float(np.asarray(max_ts))

    # max polarity-group size across the batch, padded to a multiple of
    # 128*U (loop unroll granularity)
    p = event_list[:, :, 3]
    maxcnt = max(int((p > 0).sum(axis=1).max()), int((p <= 0).sum(axis=1).max()))
    quantum = P * U
    NP = ((maxcnt + quantum - 1) // quantum) * quantum
    C = NP // P

    key = (C, mt, B)
    nc = _NC_CACHE.get(key)
    if nc is None:
        nc = _build(C, mt, num_devices=B)
        _NC_CACHE[key] = nc

    in_maps = _pack_inputs(event_list, flow, vector_list, NP)
    res = run_bass_kernel_spmd(nc, in_maps, core_ids=list(range(B)), **_RUN_KWARGS)
    _LAST_RESULT = res
    vals = np.array(
        [res.results[b]["loss"][0, 0] for b in range(B)], dtype=np.float32
    )
    return np.float32(np.sum(vals, dtype=np.float32))


# revision 21
# speedup vs baseline: 1.0234x; 1.0234x over previous
"""Trainium2 Bass kernel for nn_EventWarping (contrast-maximization event
warping loss).

Strategy (data-parallel over batch, one NeuronCore per batch element):
  The core op is a bilinear scatter-add of N=262144 warped events into a
  256x256 image (4 images per warp: pos/neg polarity x {weight, weight*ts}).
  We use the TensorEngine outer-product histogram: for a chunk of 128
  events build per-event y-tent rows (lhsT) and x-tent rows (rhs) and
  accumulate image += lhsT^T @ rhs into PSUM.

  v2 over the original baseline:
  - Events are split by polarity on the host, so each chunk feeds exactly
    one polarity's images: 4 matmuls per chunk instead of 8.
  - Tents are built as t(d) = relu(1 - |iota - w|) directly from the raw
    warped coordinate (no floor/frac preprocessing on the critical path).
    We actually compute the NEGATED tent  tn = min(|iota - w|, 1) - 1  in a
    single fused tensor_scalar; lhsT and rhs are both negated so the
    matmul products are unchanged.
  - |iota - w| comes from the Scalar engine (Abs activation with a
    per-partition f32 bias) for 3 of the 4 warp-coords, and from a
    bf16 int/frac broadcast-subtract chain on DVE+GpSimd for the 4th,
    balancing the three element-wise engines.
  - 16-chunk unrolled hardware loop (the For_i cross-engine barrier costs
    ~1.6us per iteration).
  Epilogue computes sum((num/(den+eps))^2)/mt^2/nonzero_px per warp plus
  the Charbonnier flow-smoothness term on-device; host sums the 8
  per-core partial losses.
"""

import sys

if "/opt/trn_rl_repo" not in sys.path:
    sys.path.insert(0, "/opt/trn_rl_repo")

from contextlib import ExitStack

import ml_dtypes
import numpy as np

import concourse.bacc as bacc
import concourse.bass as bass
import concourse.mybir as mybir
from concourse.tile import TileContext

F32 = mybir.dt.float32
BF16 = mybir.dt.bfloat16
I32 = mybir.dt.int32
AL = mybir.AluOpType
ACTF = mybir.ActivationFunctionType

P = 128
RES = 256
NPIX = RES * RES
EPS = 1e-9
FLOW_TEMP_REG = 1e-3
import os as _os
U = int(_os.environ.get("EVW_U", "16"))  # chunks per hardware-loop iteration
_E1E2_ENGINE = _os.environ.get("EVW_E1E2", "gpsimd")
CB = 512  # static block size (dynamic AP offsets are register-limited)
PAD_POS = 1.0e6  # padding sentinel coordinate (far outside the grid)


def _emit(tc, ev, iotas, vecb, loss_out, C, mt, img_out=None):
    """C = chunks per polarity group; ev rows 5g+k hold group g's field k
    (k: 0=ts 1=y 2=x 3=fy 4=fx), each [1, C*128]."""
    nc = tc.nc
    stk = ExitStack()

    const_pool = stk.enter_context(tc.tile_pool(name="const", bufs=1))
    iota = const_pool.tile([P, 256], BF16)
    nc.sync.dma_start(iota, iotas[:, 0:256])
    ones = const_pool.tile([P, 1], F32)
    nc.gpsimd.memset(ones, 1.0)
    zk = const_pool.tile([1, 640], BF16)
    nc.gpsimd.memset(zk, 0.0)
    vtile = const_pool.tile([1, 32], F32)
    nc.sync.dma_start(vtile, vecb)

    # ---- preprocessing: per group, per warp ----
    # persistent per-(group,warp) tiles:
    #   nwy (f32)   : -warped_y            (ACT Abs bias)
    #   tw  (bf16)  : temporal weight (ts for warp0, mt-ts for warp1)
    # warp0 x: nwx0 (f32) for ACT; warp1 x: x1b/fx1b (bf16 int+frac) for DVE/GP
    fld_pool = stk.enter_context(tc.tile_pool(name="fld", bufs=1))
    grp = []
    raw_stk = ExitStack()
    raw_pool = raw_stk.enter_context(tc.tile_pool(name="raw", bufs=1))
    for g in (0, 1):
        def load_field(k):
            t = raw_pool.tile([P, C], F32, tag=f"raw{g}{k}", name=f"raw{g}{k}")
            nc.sync.dma_start(
                t, ev[5 * g + k : 5 * g + k + 1, :].rearrange("o (p c) -> (o p) c", p=P)
            )
            return t

        ts_t, y_t, x_t, fy_t, fx_t = [load_field(k) for k in range(5)]
        q_t = raw_pool.tile([P, C], F32, tag=f"q{g}", name=f"q{g}")
        nc.vector.tensor_scalar(q_t, ts_t, -1.0, float(mt), AL.mult, AL.add)

        d = {}
        scr = raw_pool.tile([P, C], F32, tag=f"scr{g}", name=f"scr{g}")
        # warp0: w = coord + (mt-ts)*flow ; warp1: w = coord - ts*flow
        for w, (mult_t, sign) in enumerate(((q_t, 1.0), (ts_t, -1.0))):
            for cname, coord, flow in (("y", y_t, fy_t), ("x", x_t, fx_t)):
                if w == 1 and cname == "x":
                    continue  # handled below via int/frac path
                nw = fld_pool.tile([P, C], F32, tag=f"nw{g}{w}{cname}",
                                   name=f"nw{g}{w}{cname}")
                nc.vector.tensor_tensor(out=scr, in0=mult_t, in1=flow, op=AL.mult)
                if sign > 0:
                    nc.vector.tensor_tensor(out=nw, in0=scr, in1=coord, op=AL.add)
                else:
                    nc.vector.tensor_tensor(out=nw, in0=coord, in1=scr, op=AL.subtract)
                    nc.vector.tensor_scalar(nw, nw, -1.0, None, AL.mult)
                    d[f"nw{w}{cname}"] = nw
                    continue
                nc.vector.tensor_scalar(nw, nw, -1.0, None, AL.mult)
                d[f"nw{w}{cname}"] = nw
        # warp1 x: split into bf16 integer + fraction
        wx1 = raw_pool.tile([P, C], F32, tag=f"wx1{g}", name=f"wx1{g}")
        nc.vector.tensor_tensor(out=scr, in0=ts_t, in1=fx_t, op=AL.mult)
        nc.vector.tensor_tensor(out=wx1, in0=x_t, in1=scr, op=AL.subtract)
        xi = raw_pool.tile([P, C], I32, tag=f"xi{g}", name=f"xi{g}")
        nc.vector.tensor_copy(out=xi, in_=wx1)
        x0f = raw_pool.tile([P, C], F32, tag=f"x0f{g}", name=f"x0f{g}")
        nc.vector.tensor_copy(out=x0f, in_=xi)
        x1b = fld_pool.tile([P, C], BF16, tag=f"x1b{g}", name=f"x1b{g}")
        nc.vector.tensor_copy(out=x1b, in_=x0f)
        nc.vector.tensor_tensor(out=scr, in0=wx1, in1=x0f, op=AL.subtract)
        fx1b = fld_pool.tile([P, C], BF16, tag=f"fx1b{g}", name=f"fx1b{g}")
        nc.vector.tensor_copy(out=fx1b, in_=scr)
        twP = fld_pool.tile([P, 2 * C], BF16, tag=f"twP{g}", name=f"twP{g}")
        nc.vector.tensor_copy(out=twP[:, 0:C], in_=ts_t)
        nc.vector.tensor_copy(out=twP[:, C : 2 * C], in_=q_t)
        d["twP"] = twP
        d["x1b"] = x1b
        d["fx1b"] = fx1b
        grp.append(d)
    raw_stk.close()

    # ---- PSUM images: banks[g][w][h] = [den(256) | num(256)] for y-half h
    psum_pool = tc.tile_pool(name="psum", bufs=1, space="PSUM")
    psum = psum_pool.__enter__()
    banks = [
        [
            [
                psum.tile([P, 512], F32, tag=f"B{g}{w}{h}", name=f"B{g}{w}{h}")
                for h in (0, 1)
            ]
            for w in (0, 1)
        ]
        for g in (0, 1)
    ]
    zl = zk[0:1, 0:128]
    zr = zk[0:1, 128:640]
    for g in (0, 1):
        for w in (0, 1):
            for h in (0, 1):
                nc.tensor.matmul(
                    out=banks[g][w][h][:], lhsT=zl, rhs=zr, start=True, stop=False
                )

    loop_pool = stk.enter_context(tc.tile_pool(name="loop", bufs=4))

    def stage_biases(g, i, base, span):
        """ACT bias APs cannot carry dynamic offsets (silently read garbage),
        so copy this iteration's U bias columns into static staging tiles."""
        d = grp[g]
        stg = {}
        for key in ("nw0y", "nw1y", "nw0x"):
            t = loop_pool.tile([P, U], F32, tag=f"stg_{key}", name=f"stg_{key}")
            nc.vector.tensor_copy(
                out=t, in_=d[key][:, base : base + span][:, bass.ds(i, U)]
            )
            stg[key] = t
        return stg

    def chunk_produce(g, i, base, span, u, stg):
        """Producer ops (ACT + GpSimd) for chunk u of this iteration."""
        d = grp[g]

        def col(t):
            return t[:, base : base + span][:, bass.ds(i, U)][:, u : u + 1]

        # ACT: |iota - w| for warp0 y, warp1 y, warp0 x -> one contiguous tile
        ayx = loop_pool.tile([P, 1024], BF16, tag="ayx", name="ayx")
        nc.scalar.activation(
            ayx[:, 0:256], iota, ACTF.Abs, bias=stg["nw0y"][:, u : u + 1], scale=1.0
        )
        nc.scalar.activation(
            ayx[:, 256:512], iota, ACTF.Abs, bias=stg["nw1y"][:, u : u + 1], scale=1.0
        )
        nc.scalar.activation(
            ayx[:, 512:768], iota, ACTF.Abs, bias=stg["nw0x"][:, u : u + 1], scale=1.0
        )
        # GpSimd: warp1 x  e = (iota - x0) - fx  via bf16 int/frac broadcasts
        e1 = loop_pool.tile([P, 256], BF16, tag="e1", name="e1")
        nc.gpsimd.tensor_tensor(
            out=e1, in0=iota, in1=col(d["x1b"]).to_broadcast((P, 256)), op=AL.subtract
        )
        e2 = loop_pool.tile([P, 256], BF16, tag="e2", name="e2")
        nc.gpsimd.tensor_tensor(
            out=e2, in0=e1, in1=col(d["fx1b"]).to_broadcast((P, 256)), op=AL.subtract
        )
        return {"ayx": ayx, "e2": e2, "u": u}

    def chunk_consume(g, i, base, span, pr, stg):
        d = grp[g]
        u = pr["u"]

        def col(t):
            return t[:, base : base + span][:, bass.ds(i, U)][:, u : u + 1]

        # w1-x abs lands in the 4th slot of the ACT tile, then ONE DVE op
        # finishes all four tents: tn = min(d,1) - 1 (negated).
        # TQ layout: [tny0 | tny1 | r00 | r10 | r01 | r11]
        ayx = pr["ayx"]
        nc.vector.scalar_tensor_tensor(
            ayx[:, 768:1024], pr["e2"], -1.0, pr["e2"], AL.mult, AL.max
        )
        TQ = loop_pool.tile([P, 1536], BF16, tag="TQ", name="TQ")
        nc.vector.tensor_scalar(TQ[:, 0:1024], ayx, 1.0, 1.0, AL.min, AL.subtract)
        tny1 = TQ[:, 256:512]
        twp = d["twP"].rearrange("p (b c) -> p b c", b=2)
        twp = twp[:, :, base : base + span][:, :, bass.ds(i, U)][:, :, u : u + 1]
        nc.vector.tensor_tensor(
            out=TQ[:, 1024:1536],
            in0=TQ[:, 512:1024],
            in1=twp.to_broadcast((P, 2, 256)),
            op=AL.mult,
        )
        # rhs: [den | num] as two 256-col blocks 512 apart
        r0 = TQ[:, 512:1536].rearrange("p (b c) -> p b c", b=4)[:, ::2, :]
        r1 = TQ[:, 768:1536].rearrange("p (b c) -> p b c", b=3)[:, ::2, :]

        for h in (0, 1):
            nc.tensor.matmul(
                out=banks[g][0][h][:],
                lhsT=TQ[:, h * 128 : (h + 1) * 128],
                rhs=r0,
                start=False,
                stop=False,
            )
        for h in (0, 1):
            nc.tensor.matmul(
                out=banks[g][1][h][:],
                lhsT=tny1[:, h * 128 : (h + 1) * 128],
                rhs=r1,
                start=False,
                stop=False,
            )

    for g in (0, 1):
        for b in range(0, C, CB):
            span = min(CB, C - b)
            with tc.For_i(0, span, U) as i:
                stg = stage_biases(g, i, b, span)
                prev = None
                for u in range(U):
                    pr = chunk_produce(g, i, b, span, u, stg)
                    if prev is not None:
                        chunk_consume(g, i, b, span, prev, stg)
                    prev = pr
                chunk_consume(g, i, b, span, prev, stg)

    for g in (0, 1):
        for w in (0, 1):
            for h in (0, 1):
                nc.tensor.matmul(
                    out=banks[g][w][h][:], lhsT=zl, rhs=zr, start=False, stop=True
                )

    if img_out is not None:
        with tc.tile_pool(name="dump", bufs=1) as dump_pool:
            stage = dump_pool.tile([P, 512], F32)
            k = 0
            for g in (0, 1):
                for w in (0, 1):
                    for h in (0, 1):
                        nc.vector.tensor_copy(out=stage, in_=banks[g][w][h][:])
                        nc.sync.dma_start(
                            img_out[k * P : (k + 1) * P, :], stage
                        )
                        k += 1

    # ---- epilogue (same math as reference) ----
    epi_pool = stk.enter_context(tc.tile_pool(name="epi", bufs=1))
    rows = epi_pool.tile([P, 4], F32)
    den = epi_pool.tile([P, 256], F32, tag="den")
    num = epi_pool.tile([P, 256], F32, tag="num")
    rec = epi_pool.tile([P, 256], F32, tag="rec")
    for w in (0, 1):
        SQ = epi_pool.tile([P, 256], F32, tag=f"SQ{w}", name=f"SQ{w}")
        Z = epi_pool.tile([P, 256], F32, tag=f"Z{w}", name=f"Z{w}")
        nc.vector.memset(SQ, 0.0)
        nc.vector.memset(Z, 0.0)
        for h in (0, 1):
            Uh, Sh = banks[0][w][h], banks[1][w][h]
            for img in (Uh, Sh):
                nc.vector.tensor_scalar(den, img[:, 0:256], EPS, None, AL.add)
                nc.vector.reciprocal(rec, den)
                nc.vector.tensor_tensor(
                    out=num, in0=img[:, 256:512], in1=rec, op=AL.mult
                )
                nc.vector.tensor_tensor(out=num, in0=num, in1=num, op=AL.mult)
                nc.vector.tensor_tensor(out=SQ, in0=SQ, in1=num, op=AL.add)
            # nonzero-pixel count uses iwe_pos + iwe_neg
            # (only one tensor_tensor input may come from PSUM -> stage S)
            nc.vector.tensor_copy(out=rec, in_=Sh[:, 0:256])
            nc.vector.tensor_tensor(out=den, in0=Uh[:, 0:256], in1=rec, op=AL.add)
            nc.vector.tensor_scalar(den, den, 0.0, None, AL.is_equal)
            nc.vector.tensor_tensor(out=Z, in0=Z, in1=den, op=AL.add)
        nc.vector.tensor_reduce(
            out=rows[:, 2 * w : 2 * w + 1], in_=SQ, axis=mybir.AxisListType.X, op=AL.add
        )
        nc.vector.tensor_reduce(
            out=rows[:, 2 * w + 1 : 2 * w + 2],
            in_=Z,
            axis=mybir.AxisListType.X,
            op=AL.add,
        )

    psum_pool.__exit__(None, None, None)

    with tc.tile_pool(name="psum2", bufs=1, space="PSUM") as psum2:
        red = psum2.tile([1, 4], F32)
        nc.tensor.matmul(out=red[:], lhsT=ones[:], rhs=rows[:], start=True, stop=True)
        scal = epi_pool.tile([1, 4], F32)
        nc.vector.tensor_copy(out=scal, in_=red[:])

    lt = epi_pool.tile([1, 1], F32)
    nc.vector.memset(lt, 0.0)
    t1 = epi_pool.tile([1, 1], F32)
    t2 = epi_pool.tile([1, 1], F32)
    for w in (0, 1):
        # t1 = 65536 - zero_count  (the reference's +EPS is an f32 no-op here)
        nc.vector.tensor_scalar(
            t1, scal[0:1, 2 * w + 1 : 2 * w + 2], -1.0, float(NPIX), AL.mult, AL.add
        )
        nc.vector.reciprocal(t2, t1)
        nc.vector.tensor_scalar(
            t1, scal[0:1, 2 * w : 2 * w + 1], 1.0 / (mt * mt), None, AL.mult
        )
        nc.vector.scalar_tensor_tensor(lt, t1, t2, lt, AL.mult, AL.add)

    # Charbonnier temporal-smoothness on vector_list
    d24 = epi_pool.tile([1, 24], F32)
    nc.vector.tensor_tensor(
        out=d24, in0=vtile[0:1, 0:24], in1=vtile[0:1, 8:32], op=AL.subtract
    )
    epsb = epi_pool.tile([1, 1], F32)
    nc.vector.memset(epsb, EPS)
    nc.scalar.activation(d24, d24, ACTF.Square)
    nc.scalar.activation(d24, d24, ACTF.Sqrt, bias=epsb[0:1, 0:1])
    ch = epi_pool.tile([1, 1], F32)
    nc.vector.tensor_reduce(out=ch, in_=d24, axis=mybir.AxisListType.X, op=AL.add)
    nc.vector.scalar_tensor_tensor(lt, ch, FLOW_TEMP_REG / 24.0, lt, AL.mult, AL.add)

    nc.sync.dma_start(loss_out, lt[:])
    stk.close()


def _build(C, mt, num_devices=8, dump_images=False):
    nc = bacc.Bacc(
        "TRN2", target_bir_lowering=False, debug=False, num_devices=num_devices
    )
    ev = nc.dram_tensor("ev", [10, C * P], F32, kind="ExternalInput")
    iotas = nc.dram_tensor("iotas", [P, 256], BF16, kind="ExternalInput")
    vecb = nc.dram_tensor("vecb", [1, 32], F32, kind="ExternalInput")
    loss = nc.dram_tensor("loss", [1, 1], F32, kind="ExternalOutput")
    img = (
        nc.dram_tensor("img", [8 * P, 512], F32, kind="ExternalOutput")
        if dump_images
        else None
    )
    with TileContext(nc) as tc:
        _emit(tc, ev.ap(), iotas.ap(), vecb.ap(), loss.ap(), C, mt,
              img_out=img.ap() if img is not None else None)
    nc.compile()
    return nc


def _host_iotas():
    a = np.arange(256, dtype=np.float32)
    return np.tile(a[None, :], (P, 1)).astype(ml_dtypes.bfloat16)


def _pack_inputs(event_list, flow, vector_list, NP):
    B = event_list.shape[0]
    iot = _host_iotas()
    maps = []
    for b in range(B):
        ts = event_list[b, :, 0]
        y = event_list[b, :, 1]
        x = event_list[b, :, 2]
        p = event_list[b, :, 3]
        fy = flow[b, :, 0]
        fx = flow[b, :, 1]
        ev = np.zeros((10, NP), np.float32)
        for g, mask in enumerate((p > 0, p <= 0)):
            cnt = int(mask.sum())
            ev[5 * g + 0, :cnt] = ts[mask]
            ev[5 * g + 1, :cnt] = y[mask]
            ev[5 * g + 2, :cnt] = x[mask]
            ev[5 * g + 3, :cnt] = fy[mask]
            ev[5 * g + 4, :cnt] = fx[mask]
            # padding: coordinate far outside the grid, zero flow/weight
            ev[5 * g + 1, cnt:] = PAD_POS
            ev[5 * g + 2, cnt:] = PAD_POS
        # device rearrange views each [1, NP] row as [128, C] row-major
        # (event e -> partition e//C, column e%C), identically for all
        # fields, so events are just permuted across chunks
        maps.append({"ev": ev, "iotas": iot,
                     "vecb": np.ascontiguousarray(
                         vector_list[b].reshape(1, 32), dtype=np.float32)})
    return maps


_NC_CACHE = {}
_RUN_KWARGS = {}  # test harness may set {"trace": True, "tmpdir": ...}
_LAST_RESULT = None


def kernel(event_list, flow, pol_mask, vector_list, max_ts):
    global _LAST_RESULT
    from concourse.bass_utils import run_bass_kernel_spmd

    event_list = np.asarray(event_list)
    flow = np.asarray(flow)
    vector_list = np.asarray(vector_list)
    B, N, _ = event_list.shape
    mt = float(np.asarray(max_ts))

    # max polarity-group size across the batch, padded to a multiple of
    # 128*U (loop unroll granularity)
    p = event_list[:, :, 3]
    maxcnt = max(int((p > 0).sum(axis=1).max()), int((p <= 0).sum(axis=1).max()))
    quantum = P * U
    NP = ((maxcnt + quantum - 1) // quantum) * quantum
    C = NP // P

    key = (C, mt, B)
    nc = _NC_CACHE.get(key)
    if nc is None:
        nc = _build(C, mt, num_devices=B)
        _NC_CACHE[key] = nc

    in_maps = _pack_inputs(event_list, flow, vector_list, NP)
    res = run_bass_kernel_spmd(nc, in_maps, core_ids=list(range(B)), **_RUN_KWARGS)
    _LAST_RESULT = res
    vals = np.array(
        [res.results[b]["loss"][0, 0] for b in range(B)], dtype=np.float32
    )
    return np.float32(np.sum(vals, dtype=np.float32))


# revision 22
# speedup vs baseline: 1.0778x; 1.0532x over previous
"""Trainium2 Bass kernel for nn_EventWarping (contrast-maximization event
warping loss).

Strategy (data-parallel over batch, one NeuronCore per batch element):
  The core op is a bilinear scatter-add of N=262144 warped events into a
  256x256 image (4 images per warp: pos/neg polarity x {weight, weight*ts}).
  We use the TensorEngine outer-product histogram: for a chunk of 128
  events build per-event y-tent rows (lhsT) and x-tent rows (rhs) and
  accumulate image += lhsT^T @ rhs into PSUM.

  v2 over the original baseline:
  - Events are split by polarity on the host, so each chunk feeds exactly
    one polarity's images: 4 matmuls per chunk instead of 8.
  - Tents are built as t(d) = relu(1 - |iota - w|) directly from the raw
    warped coordinate (no floor/frac preprocessing on the critical path).
    We actually compute the NEGATED tent  tn = min(|iota - w|, 1) - 1  in a
    single fused tensor_scalar; lhsT and rhs are both negated so the
    matmul products are unchanged.
  - |iota - w| comes from the Scalar engine (Abs activation with a
    per-partition f32 bias) for 3 of the 4 warp-coords, and from a
    bf16 int/frac broadcast-subtract chain on DVE+GpSimd for the 4th,
    balancing the three element-wise engines.
  - 16-chunk unrolled hardware loop (the For_i cross-engine barrier costs
    ~1.6us per iteration).
  Epilogue computes sum((num/(den+eps))^2)/mt^2/nonzero_px per warp plus
  the Charbonnier flow-smoothness term on-device; host sums the 8
  per-core partial losses.
"""

import sys

if "/opt/trn_rl_repo" not in sys.path:
    sys.path.insert(0, "/opt/trn_rl_repo")

from contextlib import ExitStack

import ml_dtypes
import numpy as np

import concourse.bacc as bacc
import concourse.bass as bass
import concourse.mybir as mybir
from concourse.tile import TileContext

F32 = mybir.dt.float32
BF16 = mybir.dt.bfloat16
I32 = mybir.dt.int32
AL = mybir.AluOpType
ACTF = mybir.ActivationFunctionType

P = 128
RES = 256
NPIX = RES * RES
EPS = 1e-9
FLOW_TEMP_REG = 1e-3
import os as _os
U = int(_os.environ.get("EVW_U", "16"))  # chunks per hardware-loop iteration
_E1E2_ENGINE = _os.environ.get("EVW_E1E2", "gpsimd")
CB = 512  # static block size (dynamic AP offsets are register-limited)
PAD_POS = 1.0e6  # padding sentinel coordinate (far outside the grid)


def _emit(tc, ev, iotas, vecb, loss_out, C, mt, img_out=None):
    """C = chunks per polarity group; ev rows 5g+k hold group g's field k
    (k: 0=ts 1=y 2=x 3=fy 4=fx), each [1, C*128]."""
    nc = tc.nc
    stk = ExitStack()

    const_pool = stk.enter_context(tc.tile_pool(name="const", bufs=1))
    iota = const_pool.tile([P, 256], BF16)
    nc.sync.dma_start(iota, iotas[:, 0:256])
    ones = const_pool.tile([P, 1], F32)
    nc.gpsimd.memset(ones, 1.0)
    zk = const_pool.tile([1, 640], BF16)
    nc.gpsimd.memset(zk, 0.0)
    vtile = const_pool.tile([1, 32], F32)
    nc.sync.dma_start(vtile, vecb)

    # ---- preprocessing: per group, per warp ----
    # persistent per-(group,warp) tiles:
    #   nwy (f32)   : -warped_y            (ACT Abs bias)
    #   tw  (bf16)  : temporal weight (ts for warp0, mt-ts for warp1)
    # warp0 x: nwx0 (f32) for ACT; warp1 x: x1b/fx1b (bf16 int+frac) for DVE/GP
    fld_pool = stk.enter_context(tc.tile_pool(name="fld", bufs=1))
    grp = []
    raw_stk = ExitStack()
    raw_pool = raw_stk.enter_context(tc.tile_pool(name="raw", bufs=1))
    for g in (0, 1):
        def load_field(k):
            t = raw_pool.tile([P, C], F32, tag=f"raw{g}{k}", name=f"raw{g}{k}")
            nc.sync.dma_start(
                t, ev[5 * g + k : 5 * g + k + 1, :].rearrange("o (p c) -> (o p) c", p=P)
            )
            return t

        ts_t, y_t, x_t, fy_t, fx_t = [load_field(k) for k in range(5)]
        q_t = raw_pool.tile([P, C], F32, tag=f"q{g}", name=f"q{g}")
        nc.vector.tensor_scalar(q_t, ts_t, -1.0, float(mt), AL.mult, AL.add)

        d = {}
        scr = raw_pool.tile([P, C], F32, tag=f"scr{g}", name=f"scr{g}")
        # warp0: w = coord + (mt-ts)*flow ; warp1: w = coord - ts*flow
        for w, (mult_t, sign) in enumerate(((q_t, 1.0), (ts_t, -1.0))):
            for cname, coord, flow in (("y", y_t, fy_t), ("x", x_t, fx_t)):
                if w == 1 and cname == "x":
                    continue  # handled below via int/frac path
                nw = fld_pool.tile([P, C], F32, tag=f"nw{g}{w}{cname}",
                                   name=f"nw{g}{w}{cname}")
                nc.vector.tensor_tensor(out=scr, in0=mult_t, in1=flow, op=AL.mult)
                if sign > 0:
                    nc.vector.tensor_tensor(out=nw, in0=scr, in1=coord, op=AL.add)
                else:
                    nc.vector.tensor_tensor(out=nw, in0=coord, in1=scr, op=AL.subtract)
                    nc.vector.tensor_scalar(nw, nw, -1.0, None, AL.mult)
                    d[f"nw{w}{cname}"] = nw
                    continue
                nc.vector.tensor_scalar(nw, nw, -1.0, None, AL.mult)
                d[f"nw{w}{cname}"] = nw
        # warp1 x: split into bf16 integer + fraction
        wx1 = raw_pool.tile([P, C], F32, tag=f"wx1{g}", name=f"wx1{g}")
        nc.vector.tensor_tensor(out=scr, in0=ts_t, in1=fx_t, op=AL.mult)
        nc.vector.tensor_tensor(out=wx1, in0=x_t, in1=scr, op=AL.subtract)
        xi = raw_pool.tile([P, C], I32, tag=f"xi{g}", name=f"xi{g}")
        nc.vector.tensor_copy(out=xi, in_=wx1)
        x0f = raw_pool.tile([P, C], F32, tag=f"x0f{g}", name=f"x0f{g}")
        nc.vector.tensor_copy(out=x0f, in_=xi)
        x1b = fld_pool.tile([P, C], BF16, tag=f"x1b{g}", name=f"x1b{g}")
        nc.vector.tensor_copy(out=x1b, in_=x0f)
        nc.vector.tensor_tensor(out=scr, in0=wx1, in1=x0f, op=AL.subtract)
        fx1b = fld_pool.tile([P, C], BF16, tag=f"fx1b{g}", name=f"fx1b{g}")
        nc.vector.tensor_copy(out=fx1b, in_=scr)
        twP = fld_pool.tile([P, 2 * C], BF16, tag=f"twP{g}", name=f"twP{g}")
        nc.vector.tensor_copy(out=twP[:, 0:C], in_=ts_t)
        nc.vector.tensor_copy(out=twP[:, C : 2 * C], in_=q_t)
        d["twP"] = twP
        d["x1b"] = x1b
        d["fx1b"] = fx1b
        grp.append(d)
    raw_stk.close()

    # ---- PSUM images: banks[g][w][h] = [den(256) | num(256)] for y-half h
    psum_pool = tc.tile_pool(name="psum", bufs=1, space="PSUM")
    psum = psum_pool.__enter__()
    banks = [
        [
            [
                psum.tile([P, 512], F32, tag=f"B{g}{w}{h}", name=f"B{g}{w}{h}")
                for h in (0, 1)
            ]
            for w in (0, 1)
        ]
        for g in (0, 1)
    ]
    zl = zk[0:1, 0:128]
    zr = zk[0:1, 128:640]
    for g in (0, 1):
        for w in (0, 1):
            for h in (0, 1):
                nc.tensor.matmul(
                    out=banks[g][w][h][:], lhsT=zl, rhs=zr, start=True, stop=False
                )

    loop_pool = stk.enter_context(tc.tile_pool(name="loop", bufs=4))

    def stage_biases(g, i, base, span):
        """ACT bias APs cannot carry dynamic offsets (silently read garbage),
        so copy this iteration's U bias columns into static staging tiles."""
        d = grp[g]
        stg = {}
        for key in ("nw0y", "nw1y", "nw0x"):
            t = loop_pool.tile([P, U], F32, tag=f"stg_{key}", name=f"stg_{key}")
            nc.vector.tensor_copy(
                out=t, in_=d[key][:, base : base + span][:, bass.ds(i, U)]
            )
            stg[key] = t
        return stg

    def chunk_produce(g, i, base, span, u, stg):
        """Producer ops (ACT + GpSimd) for chunk u of this iteration."""
        d = grp[g]

        def col(t):
            return t[:, base : base + span][:, bass.ds(i, U)][:, u : u + 1]

        # ACT: |iota - w| for warp0 y, warp1 y, warp0 x -> one contiguous tile
        ayx = loop_pool.tile([P, 1024], BF16, tag="ayx", name="ayx")
        nc.scalar.activation(
            ayx[:, 0:256], iota, ACTF.Abs, bias=stg["nw0y"][:, u : u + 1], scale=1.0
        )
        nc.scalar.activation(
            ayx[:, 256:512], iota, ACTF.Abs, bias=stg["nw1y"][:, u : u + 1], scale=1.0
        )
        nc.scalar.activation(
            ayx[:, 512:768], iota, ACTF.Abs, bias=stg["nw0x"][:, u : u + 1], scale=1.0
        )
        # GpSimd: warp1 x  e = (iota - x0) - fx  via bf16 int/frac broadcasts
        e1 = loop_pool.tile([P, 256], BF16, tag="e1", name="e1")
        nc.gpsimd.tensor_tensor(
            out=e1, in0=iota, in1=col(d["x1b"]).to_broadcast((P, 256)), op=AL.subtract
        )
        e2 = loop_pool.tile([P, 256], BF16, tag="e2", name="e2")
        nc.gpsimd.tensor_tensor(
            out=e2, in0=e1, in1=col(d["fx1b"]).to_broadcast((P, 256)), op=AL.subtract
        )
        return {"ayx": ayx, "e2": e2, "u": u}

    def chunk_consume(g, i, base, span, pr, stg):
        d = grp[g]
        u = pr["u"]

        def col(t):
            return t[:, base : base + span][:, bass.ds(i, U)][:, u : u + 1]

        # w1-x abs lands in the 4th slot of the ACT tile, then ONE DVE op
        # finishes all four tents: tn = min(d,1) - 1 (negated).
        # TQ layout: [tny0 | tny1 | r00 | r10 | r01 | r11]
        ayx = pr["ayx"]
        nc.vector.scalar_tensor_tensor(
            ayx[:, 768:1024], pr["e2"], -1.0, pr["e2"], AL.mult, AL.max
        )
        TQ = loop_pool.tile([P, 1536], BF16, tag="TQ", name="TQ")
        nc.vector.tensor_scalar(TQ[:, 0:1024], ayx, 1.0, 1.0, AL.min, AL.subtract)
        tny1 = TQ[:, 256:512]
        twp = d["twP"].rearrange("p (b c) -> p b c", b=2)
        twp = twp[:, :, base : base + span][:, :, bass.ds(i, U)][:, :, u : u + 1]
        nc.vector.tensor_tensor(
            out=TQ[:, 1024:1536],
            in0=TQ[:, 512:1024],
            in1=twp.to_broadcast((P, 2, 256)),
            op=AL.mult,
        )
        # rhs: [den | num] as two 256-col blocks 512 apart
        r0 = TQ[:, 512:1536].rearrange("p (b c) -> p b c", b=4)[:, ::2, :]
        r1 = TQ[:, 768:1536].rearrange("p (b c) -> p b c", b=3)[:, ::2, :]

        for h in (0, 1):
            nc.tensor.matmul(
                out=banks[g][0][h][:],
                lhsT=TQ[:, h * 128 : (h + 1) * 128],
                rhs=r0,
                start=False,
                stop=False,
            )
        for h in (0, 1):
            nc.tensor.matmul(
                out=banks[g][1][h][:],
                lhsT=tny1[:, h * 128 : (h + 1) * 128],
                rhs=r1,
                start=False,
                stop=False,
            )

    for g in (0, 1):
        for b in range(0, C, CB):
            span = min(CB, C - b)
            with tc.For_i(0, span, U) as i:
                # keepalive: zero-weight matmuls bridge the PE's matmul
                # drought across the iteration boundary (barrier + first
                # produce), else HAM re-throttles the PE clock every
                # iteration (measured 257 warm/cold oscillations).
                for w in (0, 1):
                    nc.tensor.matmul(
                        out=banks[g][w][0][:], lhsT=zl, rhs=zr,
                        start=False, stop=False,
                    )
                stg = stage_biases(g, i, b, span)
                prev = None
                for u in range(U):
                    pr = chunk_produce(g, i, b, span, u, stg)
                    if prev is not None:
                        chunk_consume(g, i, b, span, prev, stg)
                    prev = pr
                chunk_consume(g, i, b, span, prev, stg)

    for g in (0, 1):
        for w in (0, 1):
            for h in (0, 1):
                nc.tensor.matmul(
                    out=banks[g][w][h][:], lhsT=zl, rhs=zr, start=False, stop=True
                )

    if img_out is not None:
        with tc.tile_pool(name="dump", bufs=1) as dump_pool:
            stage = dump_pool.tile([P, 512], F32)
            k = 0
            for g in (0, 1):
                for w in (0, 1):
                    for h in (0, 1):
                        nc.vector.tensor_copy(out=stage, in_=banks[g][w][h][:])
                        nc.sync.dma_start(
                            img_out[k * P : (k + 1) * P, :], stage
                        )
                        k += 1

    # ---- epilogue (same math as reference) ----
    epi_pool = stk.enter_context(tc.tile_pool(name="epi", bufs=1))
    rows = epi_pool.tile([P, 4], F32)
    den = epi_pool.tile([P, 256], F32, tag="den")
    num = epi_pool.tile([P, 256], F32, tag="num")
    rec = epi_pool.tile([P, 256], F32, tag="rec")
    for w in (0, 1):
        SQ = epi_pool.tile([P, 256], F32, tag=f"SQ{w}", name=f"SQ{w}")
        Z = epi_pool.tile([P, 256], F32, tag=f"Z{w}", name=f"Z{w}")
        nc.vector.memset(SQ, 0.0)
        nc.vector.memset(Z, 0.0)
        for h in (0, 1):
            Uh, Sh = banks[0][w][h], banks[1][w][h]
            for img in (Uh, Sh):
                nc.vector.tensor_scalar(den, img[:, 0:256], EPS, None, AL.add)
                nc.vector.reciprocal(rec, den)
                nc.vector.tensor_tensor(
                    out=num, in0=img[:, 256:512], in1=rec, op=AL.mult
                )
                nc.vector.tensor_tensor(out=num, in0=num, in1=num, op=AL.mult)
                nc.vector.tensor_tensor(out=SQ, in0=SQ, in1=num, op=AL.add)
            # nonzero-pixel count uses iwe_pos + iwe_neg
            # (only one tensor_tensor input may come from PSUM -> stage S)
            nc.vector.tensor_copy(out=rec, in_=Sh[:, 0:256])
            nc.vector.tensor_tensor(out=den, in0=Uh[:, 0:256], in1=rec, op=AL.add)
            nc.vector.tensor_scalar(den, den, 0.0, None, AL.is_equal)
            nc.vector.tensor_tensor(out=Z, in0=Z, in1=den, op=AL.add)
        nc.vector.tensor_reduce(
            out=rows[:, 2 * w : 2 * w + 1], in_=SQ, axis=mybir.AxisListType.X, op=AL.add
        )
        nc.vector.tensor_reduce(
            out=rows[:, 2 * w + 1 : 2 * w + 2],
            in_=Z,
            axis=mybir.AxisListType.X,
            op=AL.add,
        )

    psum_pool.__exit__(None, None, None)

    with tc.tile_pool(name="psum2", bufs=1, space="PSUM") as psum2:
        red = psum2.tile([1, 4], F32)
        nc.tensor.matmul(out=red[:], lhsT=ones[:], rhs=rows[:], start=True, stop=True)
        scal = epi_pool.tile([1, 4], F32)
        nc.vector.tensor_copy(out=scal, in_=red[:])

    lt = epi_pool.tile([1, 1], F32)
    nc.vector.memset(lt, 0.0)
    t1 = epi_pool.tile([1, 1], F32)
    t2 = epi_pool.tile([1, 1], F32)
    for w in (0, 1):
        # t1 = 65536 - zero_count  (the reference's +EPS is an f32 no-op here)
        nc.vector.tensor_scalar(
            t1, scal[0:1, 2 * w + 1 : 2 * w + 2], -1.0, float(NPIX), AL.mult, AL.add
        )
        nc.vector.reciprocal(t2, t1)
        nc.vector.tensor_scalar(
            t1, scal[0:1, 2 * w : 2 * w + 1], 1.0 / (mt * mt), None, AL.mult
        )
        nc.vector.scalar_tensor_tensor(lt, t1, t2, lt, AL.mult, AL.add)

    # Charbonnier temporal-smoothness on vector_list
    d24 = epi_pool.tile([1, 24], F32)
    nc.vector.tensor_tensor(
        out=d24, in0=vtile[0:1, 0:24], in1=vtile[0:1, 8:32], op=AL.subtract
    )
    epsb = epi_pool.tile([1, 1], F32)
    nc.vector.memset(epsb, EPS)
    nc.scalar.activation(d24, d24, ACTF.Square)
    nc.scalar.activation(d24, d24, ACTF.Sqrt, bias=epsb[0:1, 0:1])
    ch = epi_pool.tile([1, 1], F32)
    nc.vector.tensor_reduce(out=ch, in_=d24, axis=mybir.AxisListType.X, op=AL.add)
    nc.vector.scalar_tensor_tensor(lt, ch, FLOW_TEMP_REG / 24.0, lt, AL.mult, AL.add)

    nc.sync.dma_start(loss_out, lt[:])
    stk.close()


def _build(C, mt, num_devices=8, dump_images=False):
    nc = bacc.Bacc(
        "TRN2", target_bir_lowering=False, debug=False, num_devices=num_devices
    )
    ev = nc.dram_tensor("ev", [10, C * P], F32, kind="ExternalInput")
    iotas = nc.dram_tensor("iotas", [P, 256], BF16, kind="ExternalInput")
    vecb = nc.dram_tensor("vecb", [1, 32], F32, kind="ExternalInput")
    loss = nc.dram_tensor("loss", [1, 1], F32, kind="ExternalOutput")
    img = (
        nc.dram_tensor("img", [8 * P, 512], F32, kind="ExternalOutput")
        if dump_images
        else None
    )
    with TileContext(nc) as tc:
        _emit(tc, ev.ap(), iotas.ap(), vecb.ap(), loss.ap(), C, mt,
              img_out=img.ap() if img is not None else None)
    nc.compile()
    return nc


def _host_iotas():
    a = np.arange(256, dtype=np.float32)
    return np.tile(a[None, :], (P, 1)).astype(ml_dtypes.bfloat16)


def _pack_inputs(event_list, flow, vector_list, NP):
    B = event_list.shape[0]
    iot = _host_iotas()
    maps = []
    for b in range(B):
        ts = event_list[b, :, 0]
        y = event_list[b, :, 1]
        x = event_list[b, :, 2]
        p = event_list[b, :, 3]
        fy = flow[b, :, 0]
        fx = flow[b, :, 1]
        ev = np.zeros((10, NP), np.float32)
        for g, mask in enumerate((p > 0, p <= 0)):
            cnt = int(mask.sum())
            ev[5 * g + 0, :cnt] = ts[mask]
            ev[5 * g + 1, :cnt] = y[mask]
            ev[5 * g + 2, :cnt] = x[mask]
            ev[5 * g + 3, :cnt] = fy[mask]
            ev[5 * g + 4, :cnt] = fx[mask]
            # padding: coordinate far outside the grid, zero flow/weight
            ev[5 * g + 1, cnt:] = PAD_POS
            ev[5 * g + 2, cnt:] = PAD_POS
        # device rearrange views each [1, NP] row as [128, C] row-major
        # (event e -> partition e//C, column e%C), identically for all
        # fields, so events are just permuted across chunks
        maps.append({"ev": ev, "iotas": iot,
                     "vecb": np.ascontiguousarray(
                         vector_list[b].reshape(1, 32), dtype=np.float32)})
    return maps


_NC_CACHE = {}
_RUN_KWARGS = {}  # test harness may set {"trace": True, "tmpdir": ...}
_LAST_RESULT = None


def kernel(event_list, flow, pol_mask, vector_list, max_ts):
    global _LAST_RESULT
    from concourse.bass_utils import run_bass_kernel_spmd

    event_list = np.asarray(event_list)
    flow = np.asarray(flow)
    vector_list = np.asarray(vector_list)
    B, N, _ = event_list.shape
    mt = float(np.asarray(max_ts))

    # max polarity-group size across the batch, padded to a multiple of
    # 128*U (loop unroll granularity)
    p = event_list[:, :, 3]
    maxcnt = max(int((p > 0).sum(axis=1).max()), int((p <= 0).sum(axis=1).max()))
    quantum = P * U
    NP = ((maxcnt + quantum - 1) // quantum) * quantum
    C = NP // P

    key = (C, mt, B)
    nc = _NC_CACHE.get(key)
    if nc is None:
        nc = _build(C, mt, num_devices=B)
        _NC_CACHE[key] = nc

    in_maps = _pack_inputs(event_list, flow, vector_list, NP)
    res = run_bass_kernel_spmd(nc, in_maps, core_ids=list(range(B)), **_RUN_KWARGS)
    _LAST_RESULT = res
    vals = np.array(
        [res.results[b]["loss"][0, 0] for b in range(B)], dtype=np.float32
    )
    return np.float32(np.sum(vals, dtype=np.float32))


# revision 23
# speedup vs baseline: 1.0960x; 1.0169x over previous
"""Trainium2 Bass kernel for nn_EventWarping (contrast-maximization event
warping loss).

Strategy (data-parallel over batch, one NeuronCore per batch element):
  The core op is a bilinear scatter-add of N=262144 warped events into a
  256x256 image (4 images per warp: pos/neg polarity x {weight, weight*ts}).
  We use the TensorEngine outer-product histogram: for a chunk of 128
  events build per-event y-tent rows (lhsT) and x-tent rows (rhs) and
  accumulate image += lhsT^T @ rhs into PSUM.

  v2 over the original baseline:
  - Events are split by polarity on the host, so each chunk feeds exactly
    one polarity's images: 4 matmuls per chunk instead of 8.
  - Tents are built as t(d) = relu(1 - |iota - w|) directly from the raw
    warped coordinate (no floor/frac preprocessing on the critical path).
    We actually compute the NEGATED tent  tn = min(|iota - w|, 1) - 1  in a
    single fused tensor_scalar; lhsT and rhs are both negated so the
    matmul products are unchanged.
  - |iota - w| comes from the Scalar engine (Abs activation with a
    per-partition f32 bias) for 3 of the 4 warp-coords, and from a
    bf16 int/frac broadcast-subtract chain on DVE+GpSimd for the 4th,
    balancing the three element-wise engines.
  - 16-chunk unrolled hardware loop (the For_i cross-engine barrier costs
    ~1.6us per iteration).
  Epilogue computes sum((num/(den+eps))^2)/mt^2/nonzero_px per warp plus
  the Charbonnier flow-smoothness term on-device; host sums the 8
  per-core partial losses.
"""

import sys

if "/opt/trn_rl_repo" not in sys.path:
    sys.path.insert(0, "/opt/trn_rl_repo")

from contextlib import ExitStack

import ml_dtypes
import numpy as np

import concourse.bacc as bacc
import concourse.bass as bass
import concourse.mybir as mybir
from concourse.tile import TileContext

F32 = mybir.dt.float32
BF16 = mybir.dt.bfloat16
I32 = mybir.dt.int32
AL = mybir.AluOpType
ACTF = mybir.ActivationFunctionType

P = 128
RES = 256
NPIX = RES * RES
EPS = 1e-9
FLOW_TEMP_REG = 1e-3
import os as _os
U = int(_os.environ.get("EVW_U", "16"))  # chunks per hardware-loop iteration
_E1E2_ENGINE = _os.environ.get("EVW_E1E2", "gpsimd")
CB = 512  # static block size (dynamic AP offsets are register-limited)
PAD_POS = 1.0e6  # padding sentinel coordinate (far outside the grid)


def _emit(tc, ev, iotas, vecb, loss_out, C, mt, img_out=None):
    """C = chunks per polarity group; ev rows 5g+k hold group g's field k
    (k: 0=ts 1=y 2=x 3=fy 4=fx), each [1, C*128]."""
    nc = tc.nc
    stk = ExitStack()

    const_pool = stk.enter_context(tc.tile_pool(name="const", bufs=1))
    iota = const_pool.tile([P, 256], BF16)
    nc.sync.dma_start(iota, iotas[:, 0:256])
    ones = const_pool.tile([P, 1], F32)
    nc.gpsimd.memset(ones, 1.0)
    zk = const_pool.tile([1, 640], BF16)
    nc.gpsimd.memset(zk, 0.0)
    vtile = const_pool.tile([1, 32], F32)
    nc.sync.dma_start(vtile, vecb)

    # ---- preprocessing: per group, per warp ----
    # persistent per-(group,warp) tiles:
    #   nwy (f32)   : -warped_y            (ACT Abs bias)
    #   tw  (bf16)  : temporal weight (ts for warp0, mt-ts for warp1)
    # warp0 x: nwx0 (f32) for ACT; warp1 x: x1b/fx1b (bf16 int+frac) for DVE/GP
    fld_pool = stk.enter_context(tc.tile_pool(name="fld", bufs=1))
    grp = []
    raw_stk = ExitStack()
    raw_pool = raw_stk.enter_context(tc.tile_pool(name="raw", bufs=1))
    for g in (0, 1):
        def load_field(k):
            t = raw_pool.tile([P, C], F32, tag=f"raw{g}{k}", name=f"raw{g}{k}")
            nc.sync.dma_start(
                t, ev[5 * g + k : 5 * g + k + 1, :].rearrange("o (p c) -> (o p) c", p=P)
            )
            return t

        ts_t, y_t, x_t, fy_t, fx_t = [load_field(k) for k in range(5)]
        q_t = raw_pool.tile([P, C], F32, tag=f"q{g}", name=f"q{g}")
        nc.vector.tensor_scalar(q_t, ts_t, -1.0, float(mt), AL.mult, AL.add)

        d = {}
        scr = raw_pool.tile([P, C], F32, tag=f"scr{g}", name=f"scr{g}")
        # warp0: w = coord + (mt-ts)*flow ; warp1: w = coord - ts*flow
        for w, (mult_t, sign) in enumerate(((q_t, 1.0), (ts_t, -1.0))):
            for cname, coord, flow in (("y", y_t, fy_t), ("x", x_t, fx_t)):
                if w == 1 and cname == "x":
                    continue  # handled below via int/frac path
                nw = fld_pool.tile([P, C], F32, tag=f"nw{g}{w}{cname}",
                                   name=f"nw{g}{w}{cname}")
                nc.vector.tensor_tensor(out=scr, in0=mult_t, in1=flow, op=AL.mult)
                if sign > 0:
                    nc.vector.tensor_tensor(out=nw, in0=scr, in1=coord, op=AL.add)
                else:
                    nc.vector.tensor_tensor(out=nw, in0=coord, in1=scr, op=AL.subtract)
                    nc.vector.tensor_scalar(nw, nw, -1.0, None, AL.mult)
                    d[f"nw{w}{cname}"] = nw
                    continue
                nc.vector.tensor_scalar(nw, nw, -1.0, None, AL.mult)
                d[f"nw{w}{cname}"] = nw
        # warp1 x: split into bf16 integer + fraction
        wx1 = raw_pool.tile([P, C], F32, tag=f"wx1{g}", name=f"wx1{g}")
        nc.vector.tensor_tensor(out=scr, in0=ts_t, in1=fx_t, op=AL.mult)
        nc.vector.tensor_tensor(out=wx1, in0=x_t, in1=scr, op=AL.subtract)
        xi = raw_pool.tile([P, C], I32, tag=f"xi{g}", name=f"xi{g}")
        nc.vector.tensor_copy(out=xi, in_=wx1)
        x0f = raw_pool.tile([P, C], F32, tag=f"x0f{g}", name=f"x0f{g}")
        nc.vector.tensor_copy(out=x0f, in_=xi)
        x1b = fld_pool.tile([P, C], BF16, tag=f"x1b{g}", name=f"x1b{g}")
        nc.vector.tensor_copy(out=x1b, in_=x0f)
        nc.vector.tensor_tensor(out=scr, in0=wx1, in1=x0f, op=AL.subtract)
        fx1b = fld_pool.tile([P, C], BF16, tag=f"fx1b{g}", name=f"fx1b{g}")
        nc.vector.tensor_copy(out=fx1b, in_=scr)
        twP = fld_pool.tile([P, 2 * C], BF16, tag=f"twP{g}", name=f"twP{g}")
        nc.vector.tensor_copy(out=twP[:, 0:C], in_=ts_t)
        nc.vector.tensor_copy(out=twP[:, C : 2 * C], in_=q_t)
        d["twP"] = twP
        d["x1b"] = x1b
        d["fx1b"] = fx1b
        grp.append(d)
    raw_stk.close()

    # ---- PSUM images: banks[g][w][h] = [den(256) | num(256)] for y-half h
    psum_pool = tc.tile_pool(name="psum", bufs=1, space="PSUM")
    psum = psum_pool.__enter__()
    banks = [
        [
            [
                psum.tile([P, 512], F32, tag=f"B{g}{w}{h}", name=f"B{g}{w}{h}")
                for h in (0, 1)
            ]
            for w in (0, 1)
        ]
        for g in (0, 1)
    ]
    zl = zk[0:1, 0:128]
    zr = zk[0:1, 128:640]
    for g in (0, 1):
        for w in (0, 1):
            for h in (0, 1):
                nc.tensor.matmul(
                    out=banks[g][w][h][:], lhsT=zl, rhs=zr, start=True, stop=False
                )

    loop_pool = stk.enter_context(tc.tile_pool(name="loop", bufs=4))

    def stage_biases(g, i, base, span):
        """ACT bias APs cannot carry dynamic offsets (silently read garbage),
        so copy this iteration's U bias columns into static staging tiles."""
        d = grp[g]
        stg = {}
        for key in ("nw0y", "nw1y", "nw0x"):
            t = loop_pool.tile([P, U], F32, tag=f"stg_{key}", name=f"stg_{key}")
            nc.vector.tensor_copy(
                out=t, in_=d[key][:, base : base + span][:, bass.ds(i, U)]
            )
            stg[key] = t
        return stg

    def chunk_produce(g, i, base, span, u, stg):
        """Producer ops (ACT + GpSimd) for chunk u of this iteration."""
        d = grp[g]

        def col(t):
            return t[:, base : base + span][:, bass.ds(i, U)][:, u : u + 1]

        # ACT: |iota - w| for warp0 y, warp1 y, warp0 x -> one contiguous tile
        ayx = loop_pool.tile([P, 1024], BF16, tag="ayx", name="ayx")
        nc.scalar.activation(
            ayx[:, 0:256], iota, ACTF.Abs, bias=stg["nw0y"][:, u : u + 1], scale=1.0
        )
        nc.scalar.activation(
            ayx[:, 256:512], iota, ACTF.Abs, bias=stg["nw1y"][:, u : u + 1], scale=1.0
        )
        nc.scalar.activation(
            ayx[:, 512:768], iota, ACTF.Abs, bias=stg["nw0x"][:, u : u + 1], scale=1.0
        )
        # GpSimd: warp1 x  e = (iota - x0) - fx  via bf16 int/frac broadcasts
        e1 = loop_pool.tile([P, 256], BF16, tag="e1", name="e1")
        nc.gpsimd.tensor_tensor(
            out=e1, in0=iota, in1=col(d["x1b"]).to_broadcast((P, 256)), op=AL.subtract
        )
        e2 = loop_pool.tile([P, 256], BF16, tag="e2", name="e2")
        nc.gpsimd.tensor_tensor(
            out=e2, in0=e1, in1=col(d["fx1b"]).to_broadcast((P, 256)), op=AL.subtract
        )
        return {"ayx": ayx, "e2": e2, "u": u}

    def chunk_consume(g, i, base, span, pr, stg):
        d = grp[g]
        u = pr["u"]

        def col(t):
            return t[:, base : base + span][:, bass.ds(i, U)][:, u : u + 1]

        # w1-x abs lands in the 4th slot of the ACT tile, then ONE DVE op
        # finishes all four tents: tn = min(d,1) - 1 (negated).
        # TQ layout: [tny0 | tny1 | r00 | r10 | r01 | r11]
        ayx = pr["ayx"]
        nc.vector.scalar_tensor_tensor(
            ayx[:, 768:1024], pr["e2"], -1.0, pr["e2"], AL.mult, AL.max
        )
        TQ = loop_pool.tile([P, 1536], BF16, tag="TQ", name="TQ")
        nc.vector.tensor_scalar(TQ[:, 0:1024], ayx, 1.0, 1.0, AL.min, AL.subtract)
        tny1 = TQ[:, 256:512]
        twp = d["twP"].rearrange("p (b c) -> p b c", b=2)
        twp = twp[:, :, base : base + span][:, :, bass.ds(i, U)][:, :, u : u + 1]
        nc.vector.tensor_tensor(
            out=TQ[:, 1024:1536],
            in0=TQ[:, 512:1024],
            in1=twp.to_broadcast((P, 2, 256)),
            op=AL.mult,
        )
        # rhs: [den | num] as two 256-col blocks 512 apart
        r0 = TQ[:, 512:1536].rearrange("p (b c) -> p b c", b=4)[:, ::2, :]
        r1 = TQ[:, 768:1536].rearrange("p (b c) -> p b c", b=3)[:, ::2, :]

        for h in (0, 1):
            nc.tensor.matmul(
                out=banks[g][0][h][:],
                lhsT=TQ[:, h * 128 : (h + 1) * 128],
                rhs=r0,
                start=False,
                stop=False,
            )
        for h in (0, 1):
            nc.tensor.matmul(
                out=banks[g][1][h][:],
                lhsT=tny1[:, h * 128 : (h + 1) * 128],
                rhs=r1,
                start=False,
                stop=False,
            )

    for g in (0, 1):
        for b in range(0, C, CB):
            span = min(CB, C - b)
            with tc.For_i(0, span, U, staggered_reset=True) as i:
                # keepalive: zero-weight matmuls bridge the PE's matmul
                # drought across the iteration boundary (barrier + first
                # produce), else HAM re-throttles the PE clock every
                # iteration (measured 257 warm/cold oscillations).
                for w in (0, 1):
                    nc.tensor.matmul(
                        out=banks[g][w][0][:], lhsT=zl, rhs=zr,
                        start=False, stop=False,
                    )
                stg = stage_biases(g, i, b, span)
                prev = None
                for u in range(U):
                    pr = chunk_produce(g, i, b, span, u, stg)
                    if prev is not None:
                        chunk_consume(g, i, b, span, prev, stg)
                    prev = pr
                chunk_consume(g, i, b, span, prev, stg)

    for g in (0, 1):
        for w in (0, 1):
            for h in (0, 1):
                nc.tensor.matmul(
                    out=banks[g][w][h][:], lhsT=zl, rhs=zr, start=False, stop=True
                )

    if img_out is not None:
        with tc.tile_pool(name="dump", bufs=1) as dump_pool:
            stage = dump_pool.tile([P, 512], F32)
            k = 0
            for g in (0, 1):
                for w in (0, 1):
                    for h in (0, 1):
                        nc.vector.tensor_copy(out=stage, in_=banks[g][w][h][:])
                        nc.sync.dma_start(
                            img_out[k * P : (k + 1) * P, :], stage
                        )
                        k += 1

    # ---- epilogue (same math as reference) ----
    epi_pool = stk.enter_context(tc.tile_pool(name="epi", bufs=1))
    rows = epi_pool.tile([P, 4], F32)
    den = epi_pool.tile([P, 256], F32, tag="den")
    num = epi_pool.tile([P, 256], F32, tag="num")
    rec = epi_pool.tile([P, 256], F32, tag="rec")
    for w in (0, 1):
        SQ = epi_pool.tile([P, 256], F32, tag=f"SQ{w}", name=f"SQ{w}")
        Z = epi_pool.tile([P, 256], F32, tag=f"Z{w}", name=f"Z{w}")
        nc.vector.memset(SQ, 0.0)
        nc.vector.memset(Z, 0.0)
        for h in (0, 1):
            Uh, Sh = banks[0][w][h], banks[1][w][h]
            for img in (Uh, Sh):
                nc.vector.tensor_scalar(den, img[:, 0:256], EPS, None, AL.add)
                nc.vector.reciprocal(rec, den)
                nc.vector.tensor_tensor(
                    out=num, in0=img[:, 256:512], in1=rec, op=AL.mult
                )
                nc.vector.tensor_tensor(out=num, in0=num, in1=num, op=AL.mult)
                nc.vector.tensor_tensor(out=SQ, in0=SQ, in1=num, op=AL.add)
            # nonzero-pixel count uses iwe_pos + iwe_neg
            # (only one tensor_tensor input may come from PSUM -> stage S)
            nc.vector.tensor_copy(out=rec, in_=Sh[:, 0:256])
            nc.vector.tensor_tensor(out=den, in0=Uh[:, 0:256], in1=rec, op=AL.add)
            nc.vector.tensor_scalar(den, den, 0.0, None, AL.is_equal)
            nc.vector.tensor_tensor(out=Z, in0=Z, in1=den, op=AL.add)
        nc.vector.tensor_reduce(
            out=rows[:, 2 * w : 2 * w + 1], in_=SQ, axis=mybir.AxisListType.X, op=AL.add
        )
        nc.vector.tensor_reduce(
            out=rows[:, 2 * w + 1 : 2 * w + 2],
            in_=Z,
            axis=mybir.AxisListType.X,
            op=AL.add,
        )

    psum_pool.__exit__(None, None, None)

    with tc.tile_pool(name="psum2", bufs=1, space="PSUM") as psum2:
        red = psum2.tile([1, 4], F32)
        nc.tensor.matmul(out=red[:], lhsT=ones[:], rhs=rows[:], start=True, stop=True)
        scal = epi_pool.tile([1, 4], F32)
        nc.vector.tensor_copy(out=scal, in_=red[:])

    lt = epi_pool.tile([1, 1], F32)
    nc.vector.memset(lt, 0.0)
    t1 = epi_pool.tile([1, 1], F32)
    t2 = epi_pool.tile([1, 1], F32)
    for w in (0, 1):
        # t1 = 65536 - zero_count  (the reference's +EPS is an f32 no-op here)
        nc.vector.tensor_scalar(
            t1, scal[0:1, 2 * w + 1 : 2 * w + 2], -1.0, float(NPIX), AL.mult, AL.add
        )
        nc.vector.reciprocal(t2, t1)
        nc.vector.tensor_scalar(
            t1, scal[0:1, 2 * w : 2 * w + 1], 1.0 / (mt * mt), None, AL.mult
        )
        nc.vector.scalar_tensor_tensor(lt, t1, t2, lt, AL.mult, AL.add)

    # Charbonnier temporal-smoothness on vector_list
    d24 = epi_pool.tile([1, 24], F32)
    nc.vector.tensor_tensor(
        out=d24, in0=vtile[0:1, 0:24], in1=vtile[0:1, 8:32], op=AL.subtract
    )
    epsb = epi_pool.tile([1, 1], F32)
    nc.vector.memset(epsb, EPS)
    nc.scalar.activation(d24, d24, ACTF.Square)
    nc.scalar.activation(d24, d24, ACTF.Sqrt, bias=epsb[0:1, 0:1])
    ch = epi_pool.tile([1, 1], F32)
    nc.vector.tensor_reduce(out=ch, in_=d24, axis=mybir.AxisListType.X, op=AL.add)
    nc.vector.scalar_tensor_tensor(lt, ch, FLOW_TEMP_REG / 24.0, lt, AL.mult, AL.add)

    nc.sync.dma_start(loss_out, lt[:])
    stk.close()


def _build(C, mt, num_devices=8, dump_images=False):
    nc = bacc.Bacc(
        "TRN2", target_bir_lowering=False, debug=False, num_devices=num_devices
    )
    ev = nc.dram_tensor("ev", [10, C * P], F32, kind="ExternalInput")
    iotas = nc.dram_tensor("iotas", [P, 256], BF16, kind="ExternalInput")
    vecb = nc.dram_tensor("vecb", [1, 32], F32, kind="ExternalInput")
    loss = nc.dram_tensor("loss", [1, 1], F32, kind="ExternalOutput")
    img = (
        nc.dram_tensor("img", [8 * P, 512], F32, kind="ExternalOutput")
        if dump_images
        else None
    )
    with TileContext(nc) as tc:
        _emit(tc, ev.ap(), iotas.ap(), vecb.ap(), loss.ap(), C, mt,
              img_out=img.ap() if img is not None else None)
    nc.compile()
    return nc


def _host_iotas():
    a = np.arange(256, dtype=np.float32)
    return np.tile(a[None, :], (P, 1)).astype(ml_dtypes.bfloat16)


def _pack_inputs(event_list, flow, vector_list, NP):
    B = event_list.shape[0]
    iot = _host_iotas()
    maps = []
    for b in range(B):
        ts = event_list[b, :, 0]
        y = event_list[b, :, 1]
        x = event_list[b, :, 2]
        p = event_list[b, :, 3]
        fy = flow[b, :, 0]
        fx = flow[b, :, 1]
        ev = np.zeros((10, NP), np.float32)
        for g, mask in enumerate((p > 0, p <= 0)):
            cnt = int(mask.sum())
            ev[5 * g + 0, :cnt] = ts[mask]
            ev[5 * g + 1, :cnt] = y[mask]
            ev[5 * g + 2, :cnt] = x[mask]
            ev[5 * g + 3, :cnt] = fy[mask]
            ev[5 * g + 4, :cnt] = fx[mask]
            # padding: coordinate far outside the grid, zero flow/weight
            ev[5 * g + 1, cnt:] = PAD_POS
            ev[5 * g + 2, cnt:] = PAD_POS
        # device rearrange views each [1, NP] row as [128, C] row-major
        # (event e -> partition e//C, column e%C), identically for all
        # fields, so events are just permuted across chunks
        maps.append({"ev": ev, "iotas": iot,
                     "vecb": np.ascontiguousarray(
                         vector_list[b].reshape(1, 32), dtype=np.float32)})
    return maps


_NC_CACHE = {}
_RUN_KWARGS = {}  # test harness may set {"trace": True, "tmpdir": ...}
_LAST_RESULT = None


def kernel(event_list, flow, pol_mask, vector_list, max_ts):
    global _LAST_RESULT
    from concourse.bass_utils import run_bass_kernel_spmd

    event_list = np.asarray(event_list)
    flow = np.asarray(flow)
    vector_list = np.asarray(vector_list)
    B, N, _ = event_list.shape
    mt = float(np.asarray(max_ts))

    # max polarity-group size across the batch, padded to a multiple of
    # 128*U (loop unroll granularity)
    p = event_list[:, :, 3]
    maxcnt = max(int((p > 0).sum(axis=1).max()), int((p <= 0).sum(axis=1).max()))
    quantum = P * U
    NP = ((maxcnt + quantum - 1) // quantum) * quantum
    C = NP // P

    key = (C, mt, B)
    nc = _NC_CACHE.get(key)
    if nc is None:
        nc = _build(C, mt, num_devices=B)
        _NC_CACHE[key] = nc

    in_maps = _pack_inputs(event_list, flow, vector_list, NP)
    res = run_bass_kernel_spmd(nc, in_maps, core_ids=list(range(B)), **_RUN_KWARGS)
    _LAST_RESULT = res
    vals = np.array(
        [res.results[b]["loss"][0, 0] for b in range(B)], dtype=np.float32
    )
    return np.float32(np.sum(vals, dtype=np.float32))
